# revision 10
# baseline (speedup 1.0000x reference)
"""Trainium2 Bass kernel for a Mamba-based byte LLM.

Sharding: 8 cores = 2 batch groups (quads) x 4-way tensor-parallel split of
d_inner (256 channels/core). Frontend (embed + 2 convs + LNs) is sequence
sharded (512 tokens/core) then all-gathered within each quad. Each Mamba
layer: in_proj/conv/scan/out_proj channel-sharded; x_proj partials and
out_proj partials all-reduced within the quad.

Selective scan uses the hardware tensor_tensor_scan (state = a*state + b along
the free dim) in an "s-minor" layout: partition p = (dd, s) with s = p % 16,
8 d-channels x 16 states per 128-partition tile; time on the free dim.

v2: all-bf16 dataflow (weights, activations, collectives; PSUM stays f32),
full-length 2048-col scans (no chunk handoff), Softplus fusion, gpsimd
offload for dtxc/hc, software-pipelined scan block (PE replication matmuls
run LOOK tiles ahead of the DVE scans).
"""
import sys
import numpy as np

sys.path.insert(0, '/opt/trn_rl_repo')

VOCAB = 256; EMB = 384; DM = 512; DI = 1024; DS = 16; DC = 4; DTR = 32
NL = 8; BATCH = 2; L = 2048
NCOR = 8; NQ = 4; DLOC = 256          # d_inner channels per core
TC = 512                              # time chunk
EXT = 520                             # frontend token window (512 + 2*4 halo)
LLOC = 512                            # frontend tokens per core
GEN_BUFS = 20
BIG_BUFS = 14
PW_BUFS = 8
LOOK = 4                              # scan-block ry lookahead (pipeline depth)

_PROG = None
_DBG_LAYER = None     # set to an int l before first kernel() call to also dump x after layer l
_SIM_COMPAT = False   # replace Gelu/Silu with sim-implemented functions (debug only)
_NL_RUN = NL          # number of layers to emit (debug only)
_NO_COLL = False      # replace collectives with local DMA copies (debug only)


def _bf16():
    import ml_dtypes
    return ml_dtypes.bfloat16


def _f32(x):
    return np.ascontiguousarray(np.asarray(x), dtype=np.float32)


def _bf(x):
    return np.ascontiguousarray(np.asarray(x, dtype=np.float32).astype(_bf16()))


def _pack_lhsT(W):
    """W: (M, K) with M,K multiples of 128 -> flat (128, KT*MT*128),
    tile (kt, mt) at cols [(kt*MT+mt)*128 : +128], tile[p, m] = W[128mt+m, 128kt+p]."""
    M, K = W.shape
    KT, MT = K // 128, M // 128
    out = np.zeros((128, KT * MT * 128), np.float32)
    for kt in range(KT):
        for mt in range(MT):
            blk = W[mt * 128:(mt + 1) * 128, kt * 128:(kt + 1) * 128].T
            out[:, (kt * MT + mt) * 128:(kt * MT + mt + 1) * 128] = blk
    return out


def _prep(inp):
    tokens = np.asarray(inp['tokens'])
    emb = _f32(inp['emb'])
    c1w = _f32(inp['conv1_w']); c1b = _f32(inp['conv1_b'])
    g1 = _f32(inp['ln1_g']); b1 = _f32(inp['ln1_b'])
    c2w = _f32(inp['conv2_w']); c2b = _f32(inp['conv2_b'])
    g2 = _f32(inp['ln2_g']); b2 = _f32(inp['ln2_b'])
    pos = _f32(inp['pos_emb'])
    inw = _f32(inp['in_proj_w']); cw = _f32(inp['conv_w']); cb = _f32(inp['conv_b'])
    xpw = _f32(inp['x_proj_w']); dtw = _f32(inp['dt_proj_w']); dtb = _f32(inp['dt_proj_b'])
    alog = _f32(inp['A_log']); Dp = _f32(inp['D'])
    outw = _f32(inp['out_proj_w'])
    fcw = _f32(inp['fc_w']); fcb = _f32(inp['fc_b'])

    # ---- shared tensors ----
    shared = {}
    shared['ones_row'] = _bf(np.ones((1, 128), np.float32))
    fe = np.zeros((128, 28), np.float32)
    fe[:, 0] = np.arange(128); fe[:, 1] = 1.0; fe[:, 26] = 1e-5
    for mt in range(4):
        fe[:, 2 + mt] = c1b[mt * 128:(mt + 1) * 128]
        fe[:, 6 + mt] = g1[mt * 128:(mt + 1) * 128]
        fe[:, 10 + mt] = b1[mt * 128:(mt + 1) * 128]
        fe[:, 14 + mt] = c2b[mt * 128:(mt + 1) * 128]
        fe[:, 18 + mt] = g2[mt * 128:(mt + 1) * 128]
        fe[:, 22 + mt] = b2[mt * 128:(mt + 1) * 128]
    shared['fe_cols'] = fe
    shared['ones_col'] = _bf(np.ones((128, 1), np.float32))
    shared['embw'] = _bf(emb)                              # (256, 384) lhsT as-is
    # conv1: tile (tap, kt, mt): [p, idx*128+m] = c1w[128mt+m, 128kt+p, tap]
    c1f = np.zeros((128, 5 * 3 * 4 * 128), np.float32)
    for tap in range(5):
        for kt in range(3):
            for mt in range(4):
                idx = (tap * 3 + kt) * 4 + mt
                c1f[:, idx * 128:(idx + 1) * 128] = c1w[mt * 128:(mt + 1) * 128,
                                                        kt * 128:(kt + 1) * 128, tap].T
    shared['c1w_flat'] = _bf(c1f)
    c2f = np.zeros((128, 3 * 4 * 4 * 128), np.float32)
    for tap in range(3):
        for kt in range(4):
            for mt in range(4):
                idx = (tap * 4 + kt) * 4 + mt
                c2f[:, idx * 128:(idx + 1) * 128] = c2w[mt * 128:(mt + 1) * 128,
                                                        kt * 128:(kt + 1) * 128, tap].T
    shared['c2w_flat'] = _bf(c2f)
    r16 = np.zeros((128, 16 * 128), np.float32)
    ry16 = np.zeros((128, 16 * 128), np.float32)
    p_idx = np.arange(128)
    for j in range(16):
        for m in range(128):
            r16[8 * j + m // 16, j * 128 + m] = 1.0
    for j in range(16):
        for k in range(128):
            ry16[k, j * 128 + 8 * j + k // 16] = 1.0
    shared['R16'] = _bf(r16)
    shared['RY16'] = _bf(ry16)
    # repBC: lhsT patterns that broadcast dbc's B rows (32:48) / C rows (64:80)
    # to all 128 partitions (s = p % 16), contracting over the full 80 rows.
    repbc = np.zeros((128, 256), np.float32)
    for p in range(128):
        repbc[32 + p % 16, p] = 1.0
        repbc[64 + p % 16, 128 + p] = 1.0
    shared['repBC'] = _bf(repbc)
    shared['fcw_flat'] = _bf(_pack_lhsT(fcw))              # (128, 8*128)
    fcb_c = np.zeros((128, 2), np.float32)
    fcb_c[:, 0] = fcb[0:128]; fcb_c[:, 1] = fcb[128:256]
    shared['fcb_cols'] = fcb_c

    # ---- per-core tensors ----
    in_maps = []
    for core in range(NCOR):
        b, q = core // NQ, core % NQ
        my = slice(DLOC * q, DLOC * (q + 1))
        m = dict(shared)

        t0 = LLOC * q
        te = np.full((1, EXT), -1.0, np.float32)
        for i in range(EXT):
            t = t0 - 4 + i
            if 0 <= t < L:
                te[0, i] = float(tokens[b, t])
        m['tok_ext'] = _bf(te)
        fm = np.zeros((128, 516), np.float32)
        for j in range(516):
            t = t0 - 2 + j
            if 0 <= t < L:
                fm[:, j] = 1.0
        m['fe_mask'] = fm
        pl = np.zeros((128, 4 * 512), np.float32)
        for mt in range(4):
            pl[:, mt * 512:(mt + 1) * 512] = pos[t0:t0 + 512, mt * 128:(mt + 1) * 128].T
        m['pos_loc'] = pl

        inw_f = np.zeros((NL, 128, 2048), np.float32)
        outw_f = np.zeros((NL, 128, 1024), np.float32)
        xpw_f = np.zeros((NL, 128, 160), np.float32)
        dtw_f = np.zeros((NL, 128, 256), np.float32)
        lcols = np.zeros((NL, 128, 48), np.float32)
        convdiag = np.zeros((NL, 128, 1024), np.float32)
        for l in range(NL):
            Wl = np.concatenate([inw[l, my, :], inw[l, DI + DLOC * q: DI + DLOC * (q + 1), :]], 0)
            inw_f[l] = _pack_lhsT(Wl)                       # (512,512) -> (128,2048)
            outw_f[l] = _pack_lhsT(outw[l][:, my])          # (512,256) -> (128,1024)
            Wxp = np.zeros((80, DLOC), np.float32)
            Wxp[0:32] = xpw[l, 0:32, my]
            Wxp[32:48] = xpw[l, 32:48, my]
            Wxp[64:80] = xpw[l, 48:64, my]
            for kt in range(2):
                xpw_f[l][:, kt * 80:(kt + 1) * 80] = Wxp[:, kt * 128:(kt + 1) * 128].T
            Wdt = dtw[l, my, :]                             # (256, 32)
            for mt in range(2):
                dtw_f[l][0:32, mt * 128:(mt + 1) * 128] = Wdt[mt * 128:(mt + 1) * 128, :].T
            for half in range(2):
                hs = slice(half * 128, (half + 1) * 128)
                for tap in range(DC):
                    idx = half * 4 + tap
                    convdiag[l][:, idx * 128:(idx + 1) * 128] = np.diag(cw[l, my, :][hs.start:hs.stop, tap])
                lcols[l, :, 8 + half] = cb[l, my][hs]
                lcols[l, :, 10 + half] = dtb[l, my][hs]
                lcols[l, :, 44 + half] = Dp[l, my][hs]
            A = -np.exp(alog[l, my, :])                     # (256, 16)
            for g in range(32):
                lcols[l, :, 12 + g] = A[8 * g + p_idx // 16, p_idx % 16]
        m['inw_flat'] = _bf(inw_f); m['outw_flat'] = _bf(outw_f); m['xpw_flat'] = _bf(xpw_f)
        m['dtw_flat'] = _bf(dtw_f); m['lcols'] = lcols; m['convdiag'] = _bf(convdiag)
        in_maps.append(m)
    return in_maps


def _build():
    import concourse.bass as bass
    import concourse.bacc as bacc
    import concourse.mybir as mybir
    import concourse.tile as tile

    F32 = mybir.dt.float32
    BF16 = mybir.dt.bfloat16
    AF = mybir.ActivationFunctionType
    OP = mybir.AluOpType
    GROUPS = [[0, 1, 2, 3], [4, 5, 6, 7]]
    AF_GELU = AF.Tanh if _SIM_COMPAT else AF.Gelu
    AF_SILU = AF.Sigmoid if _SIM_COMPAT else AF.Silu

    nc = bacc.Bacc("TRN2", target_bir_lowering=False, debug=False, num_devices=NCOR)

    def din(name, shape, dt_=BF16):
        return nc.dram_tensor(name, list(shape), dt_, kind="ExternalInput").ap()

    t_tok = din('tok_ext', (1, EXT))
    t_ones = din('ones_row', (1, 128))
    t_onesc = din('ones_col', (128, 1))
    t_fe = din('fe_cols', (128, 28), F32)
    t_embw = din('embw', (256, 384))
    t_c1w = din('c1w_flat', (128, 7680))
    t_c2w = din('c2w_flat', (128, 6144))
    t_r16 = din('R16', (128, 2048))
    t_ry16 = din('RY16', (128, 2048))
    t_rep8 = din('repBC', (128, 256))
    t_fcw = din('fcw_flat', (128, 1024))
    t_fcb = din('fcb_cols', (128, 2), F32)
    t_pos = din('pos_loc', (128, 2048), F32)
    t_femask = din('fe_mask', (128, 516), F32)
    t_inw = din('inw_flat', (NL, 128, 2048))
    t_outw = din('outw_flat', (NL, 128, 1024))
    t_xpw = din('xpw_flat', (NL, 128, 160))
    t_dtw = din('dtw_flat', (NL, 128, 256))
    t_lcols = din('lcols', (NL, 128, 48), F32)
    t_cdiag = din('convdiag', (NL, 128, 1024))
    t_logits = nc.dram_tensor("logits", [256, 1], F32, kind="ExternalOutput").ap()
    t_xdbg = (nc.dram_tensor("xdbg", [512, L], BF16, kind="ExternalOutput").ap()
              if _DBG_LAYER is not None else None)

    with tile.TileContext(nc) as tc:
        with tc.tile_pool(name="per", bufs=1) as per, \
             tc.tile_pool(name="gen", bufs=GEN_BUFS) as gen, \
             tc.tile_pool(name="big", bufs=BIG_BUFS) as bigp, \
             tc.tile_pool(name="pw", bufs=PW_BUFS) as pw, \
             tc.tile_pool(name="lcp", bufs=2) as lcp, \
             tc.tile_pool(name="ps", bufs=4, space="PSUM") as ps, \
             tc.tile_pool(name="psy", bufs=4, space="PSUM") as psy, \
             tc.tile_pool(name="dram", bufs=1, space="DRAM") as dram:

            cnt = [0]

            def gtile(cols=TC, dt_=BF16):
                cnt[0] += 1
                return gen.tile([128, cols], dt_, tag="g", name=f"g{cnt[0]}")

            def btile():
                cnt[0] += 1
                return bigp.tile([128, L], BF16, tag="b", name=f"b{cnt[0]}")

            def pwtile(cols=512, dt_=BF16, rows=128):
                cnt[0] += 1
                return pw.tile([rows, cols], dt_, tag="w", name=f"w{cnt[0]}")

            def pstile(rows=128, cols=TC):
                cnt[0] += 1
                return ps.tile([rows, cols], F32, tag="ps", name=f"p{cnt[0]}")

            def pytile():
                cnt[0] += 1
                return psy.tile([128, TC], F32, tag="py", name=f"y{cnt[0]}")

            # ---- persistent consts ----
            tok_sb = per.tile([1, EXT], BF16, tag="tok")
            nc.sync.dma_start(tok_sb[:], t_tok[:])
            ones_sb = per.tile([1, 128], BF16, tag="ones")
            nc.sync.dma_start(ones_sb[:], t_ones[:])
            onesc_sb = per.tile([128, 1], BF16, tag="onesc")
            nc.sync.dma_start(onesc_sb[:], t_onesc[:])
            fe_sb = per.tile([128, 28], F32, tag="fec")
            nc.sync.dma_start(fe_sb[:], t_fe[:])
            r16_sb = per.tile([128, 2048], BF16, tag="r16")
            nc.sync.dma_start(r16_sb[:], t_r16[:])
            ry16_sb = per.tile([128, 2048], BF16, tag="ry16")
            nc.sync.dma_start(ry16_sb[:], t_ry16[:])
            rep8_sb = per.tile([128, 256], BF16, tag="rep8")
            nc.sync.dma_start(rep8_sb[:], t_rep8[:])
            fcb_sb = per.tile([128, 2], F32, tag="fcb")
            nc.sync.dma_start(fcb_sb[:], t_fcb[:])
            fem_sb = per.tile([128, 516], F32, tag="fem")
            nc.sync.dma_start(fem_sb[:], t_femask[:])

            iota = fe_sb[:, 0:1]

            # ---- persistent big tensors ----
            x_sb = per.tile([128, 4 * L], BF16, tag="xsb")         # K-tile kt at cols kt*L
            xi_sb = per.tile([128, 2 * (L + 3)], BF16, tag="xi")   # half at cols h*(L+3), 3-zero pad
            z3_sb = per.tile([128, 4], BF16, tag="z3")
            nc.vector.memset(z3_sb[:], 0.0)
            nc.scalar.copy(xi_sb[:, 0:3], z3_sb[:, 0:3])
            nc.scalar.copy(xi_sb[:, L + 3: L + 6], z3_sb[:, 0:3])
            zs_sb = per.tile([128, 2 * L], BF16, tag="zs")          # silu(z)
            xc_sb = per.tile([128, 2 * L], BF16, tag="xc")
            dt_sb = per.tile([128, 2 * L], BF16, tag="dt")
            dbc_sb = per.tile([80, L], BF16, tag="dbc")
            brep_sb = per.tile([128, L], BF16, tag="brep")
            crep_sb = per.tile([128, L], BF16, tag="crep")

            # ---- DRAM bounces ----
            x0b = dram.tile([512, 512], BF16, tag="x0b")
            xg = dram.tile([4, 512, 512], BF16, tag="xg")
            dbcb = dram.tile([80, L], BF16, tag="dbcb")
            dbcr = dram.tile([80, L], BF16, tag="dbcr")
            xarb = [dram.tile([512, TC], BF16, tag=f"xarb{c}", name=f"xarb{c}")
                    for c in range(4)]
            xaro = [dram.tile([512, TC], BF16, tag=f"xaro{c}", name=f"xaro{c}")
                    for c in range(4)]

            # =================== FRONTEND ===================
            # one-hot: (256, 520) as 2 row-tiles
            oh = []
            for kt in range(2):
                oht = gtile(EXT, BF16)
                for nch in range(2):
                    ptok = pstile(128, 260)
                    nc.tensor.matmul(ptok[:], ones_sb[:], tok_sb[:, nch * 260:(nch + 1) * 260],
                                     start=True, stop=True)
                    if kt == 0:
                        nc.vector.tensor_scalar(oht[:, nch * 260:(nch + 1) * 260],
                                                ptok[:], iota, None, OP.is_equal)
                    else:
                        nc.vector.tensor_scalar(oht[:, nch * 260:(nch + 1) * 260],
                                                ptok[:], 128.0, iota, OP.subtract, OP.is_equal)
                oh.append(oht)
            # embedding: x_emb (384, 520) = embw.T @ oh
            embw_sb = []
            for kt in range(2):
                et = pwtile(384)
                nc.sync.dma_start(et[:], t_embw[kt * 128:(kt + 1) * 128, :])
                embw_sb.append(et)
            xemb = []
            for mt in range(3):
                xt = gtile(EXT, BF16)
                for nch in range(2):
                    pe = pstile(128, 260)
                    for kt in range(2):
                        nc.tensor.matmul(pe[:], embw_sb[kt][:, mt * 128:(mt + 1) * 128],
                                         oh[kt][:, nch * 260:(nch + 1) * 260],
                                         start=(kt == 0), stop=(kt == 1))
                    nc.scalar.copy(xt[:, nch * 260:(nch + 1) * 260], pe[:])
                xemb.append(xt)

            def fe_conv(src, KT, TAPS, wflat, bias_base, n_out, src_off):
                """conv + gelu: out tiles (128, n_out) list of 4. src: list of KT tiles.
                out col j corresponds to src col j + src_off + tap."""
                nhalf = n_out // 2
                outs = [gtile(EXT, BF16) for _ in range(4)]
                for nch in range(2):
                    pcs = [pstile(128, nhalf) for _ in range(4)]
                    for tap in range(TAPS):
                        for kt in range(KT):
                            base = (tap * KT + kt) * 4 * 128
                            wt = pwtile(512)
                            nc.sync.dma_start(wt[:], wflat[:, base:base + 512])
                            for mt in range(4):
                                col = src_off + tap + nch * nhalf
                                nc.tensor.matmul(pcs[mt][:], wt[:, mt * 128:(mt + 1) * 128],
                                                 src[kt][:, col:col + nhalf],
                                                 start=(tap == 0 and kt == 0),
                                                 stop=(tap == TAPS - 1 and kt == KT - 1))
                    for mt in range(4):
                        nc.scalar.activation(outs[mt][:, nch * nhalf:(nch + 1) * nhalf], pcs[mt][:],
                                             AF_GELU, bias=fe_sb[:, bias_base + mt:bias_base + mt + 1],
                                             scale=1.0)
                return outs

            # conv1: out ext range [2, 518) -> 516 cols; src col = out_col + tap (out col 0 == ext 2)
            h1 = fe_conv(xemb, 3, 5, t_c1w, 2, 516, 0)

            def fe_ln(src, n, g_base, b_base, mask=None):
                """layernorm over 512 channels (4 tiles); returns 4 tiles (128, n)."""
                nhalf = n // 2
                sq = []
                for mt in range(4):
                    st = gtile(EXT, BF16)
                    nc.scalar.activation(st[:, 0:n], src[mt][:, 0:n], AF.Square)
                    sq.append(st)
                mu = gtile(EXT, BF16)     # row 0: mean
                m2 = gtile(EXT, F32)
                var = gtile(EXT, F32)
                std = gtile(EXT, F32)
                rstd = gtile(EXT, BF16)
                for nch in range(2):
                    pm = pstile(1, nhalf)
                    pm2 = pstile(1, nhalf)
                    for mt in range(4):
                        nc.tensor.matmul(pm[:], onesc_sb[:, 0:1], src[mt][:, nch * nhalf:(nch + 1) * nhalf],
                                         start=(mt == 0), stop=(mt == 3))
                        nc.tensor.matmul(pm2[:], onesc_sb[:, 0:1], sq[mt][:, nch * nhalf:(nch + 1) * nhalf],
                                         start=(mt == 0), stop=(mt == 3))
                    sl = slice(nch * nhalf, (nch + 1) * nhalf)
                    nc.scalar.activation(mu[0:1, sl], pm[:], AF.Copy, bias=0.0, scale=1.0 / 512)
                    nc.scalar.activation(m2[0:1, sl], pm[:], AF.Square, bias=0.0, scale=1.0 / 512)
                    # var = pm2/512 - mu^2
                    nc.vector.scalar_tensor_tensor(var[0:1, sl], pm2[:], 1.0 / 512, m2[0:1, sl],
                                                   OP.mult, OP.subtract)
                    nc.scalar.activation(std[0:1, sl], var[0:1, sl], AF.Sqrt,
                                         bias=fe_sb[0:1, 26:27], scale=1.0)
                    with nc.allow_low_precision(reason="rstd for broadcast matmul"):
                        nc.vector.reciprocal(rstd[0:1, sl], std[0:1, sl])
                outs = []
                for mt in range(4):
                    ot = gtile(EXT, BF16)
                    for nch in range(2):
                        sl = slice(nch * nhalf, (nch + 1) * nhalf)
                        pmb = pstile(128, nhalf)
                        prb = pstile(128, nhalf)
                        nc.tensor.matmul(pmb[:], ones_sb[:], mu[0:1, sl], start=True, stop=True)
                        nc.tensor.matmul(prb[:], ones_sb[:], rstd[0:1, sl], start=True, stop=True)
                        d0 = gtile(EXT, F32)
                        nc.vector.tensor_tensor(d0[:, 0:nhalf], src[mt][:, sl], pmb[:], OP.subtract)
                        nc.vector.tensor_tensor(d0[:, 0:nhalf], d0[:, 0:nhalf], prb[:], OP.mult)
                        if mask is None:
                            nc.vector.tensor_scalar(ot[:, sl], d0[:, 0:nhalf],
                                                    fe_sb[:, g_base + mt:g_base + mt + 1],
                                                    fe_sb[:, b_base + mt:b_base + mt + 1],
                                                    OP.mult, OP.add)
                        else:
                            nc.vector.tensor_scalar(d0[:, 0:nhalf], d0[:, 0:nhalf],
                                                    fe_sb[:, g_base + mt:g_base + mt + 1],
                                                    fe_sb[:, b_base + mt:b_base + mt + 1],
                                                    OP.mult, OP.add)
                            nc.vector.tensor_tensor(ot[:, sl], d0[:, 0:nhalf], mask[:, sl], OP.mult)
                    outs.append(ot)
                return outs

            xn1 = fe_ln(h1, 516, 6, 10, mask=fem_sb)
            # conv2: out ext [4, 516) -> 512 cols; xn1 col c == ext 2+c; src col = out_col + 1 + tap
            h2 = fe_conv(xn1, 4, 3, t_c2w, 14, 512, 1)
            xn2 = fe_ln(h2, 512, 18, 22)
            # + pos_emb -> x0 tiles; DMA to bounce
            for mt in range(4):
                pt = pwtile(512, F32)
                nc.sync.dma_start(pt[:], t_pos[:, mt * 512:(mt + 1) * 512])
                x0t = gtile(512, BF16)
                nc.vector.tensor_tensor(x0t[:], xn2[mt][:, 0:512], pt[:], OP.add)
                nc.sync.dma_start(x0b[mt * 128:(mt + 1) * 128, :], x0t[:])
            if _NO_COLL:
                for q in range(4):
                    nc.sync.dma_start(xg[q], x0b[:])
            else:
                nc.gpsimd.collective_compute("AllGather", mybir.AluOpType.bypass,
                                             replica_groups=GROUPS, ins=[x0b.opt()], outs=[xg.opt()])
            for kt in range(4):
                for q in range(4):
                    nc.sync.dma_start(x_sb[:, kt * L + q * 512: kt * L + (q + 1) * 512],
                                      xg[q, kt * 128:(kt + 1) * 128, :])

            # =================== LAYERS ===================
            for l in range(_NL_RUN):
                lc = lcp.tile([128, 48], F32, tag="lc")
                nc.sync.dma_start(lc[:], t_lcols[l])
                inw_t = []
                for kt in range(4):
                    wt = pwtile(512)
                    nc.sync.dma_start(wt[:], t_inw[l][:, kt * 512:(kt + 1) * 512])
                    inw_t.append(wt)
                xpw_t = pwtile(160)
                nc.sync.dma_start(xpw_t[:], t_xpw[l])
                dtw_t = pwtile(256)
                nc.sync.dma_start(dtw_t[:], t_dtw[l])
                outw_t = []
                for kt in range(2):
                    wt = pwtile(512)
                    nc.sync.dma_start(wt[:], t_outw[l][:, kt * 512:(kt + 1) * 512])
                    outw_t.append(wt)

                # ---- in_proj ----
                for c in range(4):
                    for mt in range(4):
                        px = pstile()
                        for kt in range(4):
                            nc.tensor.matmul(px[:], inw_t[kt][:, mt * 128:(mt + 1) * 128],
                                             x_sb[:, kt * L + c * TC: kt * L + (c + 1) * TC],
                                             start=(kt == 0), stop=(kt == 3))
                        if mt < 2:
                            nc.scalar.copy(xi_sb[:, mt * (L + 3) + 3 + c * TC: mt * (L + 3) + 3 + (c + 1) * TC],
                                           px[:])
                        else:
                            nc.scalar.activation(zs_sb[:, (mt - 2) * L + c * TC: (mt - 2) * L + (c + 1) * TC],
                                                 px[:], AF_SILU)

                # ---- causal depthwise conv (PE diag matmuls) + silu ----
                cdiag_t = []
                for half in range(2):
                    ct = pwtile(512)
                    nc.sync.dma_start(ct[:], t_cdiag[l][:, half * 512:(half + 1) * 512])
                    cdiag_t.append(ct)
                for half in range(2):
                    for c in range(4):
                        base = half * (L + 3) + c * TC
                        pcv = pstile()
                        for tap in range(DC):
                            nc.tensor.matmul(pcv[:], cdiag_t[half][:, tap * 128:(tap + 1) * 128],
                                             xi_sb[:, base + tap: base + tap + TC],
                                             start=(tap == 0), stop=(tap == DC - 1))
                        nc.scalar.activation(xc_sb[:, half * L + c * TC: half * L + (c + 1) * TC],
                                             pcv[:], AF_SILU,
                                             bias=lc[:, 8 + half: 9 + half], scale=1.0)

                # ---- x_proj partial -> DRAM -> AllReduce ----
                for c in range(4):
                    pd = pstile(80, TC)
                    for kt in range(2):
                        nc.tensor.matmul(pd[:], xpw_t[:, kt * 80:(kt + 1) * 80],
                                         xc_sb[:, kt * L + c * TC: kt * L + (c + 1) * TC],
                                         start=(kt == 0), stop=(kt == 1))
                    cnt[0] += 1
                    de = gen.tile([80, TC], BF16, tag="g", name=f"g{cnt[0]}")
                    nc.scalar.copy(de[:], pd[:])
                    nc.sync.dma_start(dbcb[:, c * TC:(c + 1) * TC], de[:])
                if _NO_COLL:
                    nc.sync.dma_start(dbcr[:], dbcb[:])
                else:
                    nc.gpsimd.collective_compute("AllReduce", mybir.AluOpType.add,
                                                 replica_groups=GROUPS, ins=[dbcb.opt()], outs=[dbcr.opt()])
                nc.sync.dma_start(dbc_sb[:], dbcr[:])

                # ---- dt = softplus(dtw @ dbc[0:32] + b) ----
                for c in range(4):
                    for mt in range(2):
                        pdt = pstile()
                        nc.tensor.matmul(pdt[:], dtw_t[0:32, mt * 128:(mt + 1) * 128],
                                         dbc_sb[0:32, c * TC:(c + 1) * TC], start=True, stop=True)
                        edt = gtile(TC, F32)
                        nc.scalar.activation(edt[:], pdt[:], AF.Exp,
                                             bias=lc[:, 10 + mt: 11 + mt], scale=1.0)
                        nc.scalar.activation(dt_sb[:, mt * L + c * TC: mt * L + (c + 1) * TC],
                                             edt[:], AF.Ln, bias=1.0, scale=1.0)

                # ---- dtxc (gpsimd), B/C broadcast for full L ----
                dtxc = []
                for half in range(2):
                    dx = btile()
                    nc.gpsimd.tensor_tensor(dx[:],
                                            dt_sb[:, half * L: (half + 1) * L],
                                            xc_sb[:, half * L: (half + 1) * L],
                                            OP.mult)
                    dtxc.append(dx)
                for c in range(4):
                    pb = pstile()
                    nc.tensor.matmul(pb[:], rep8_sb[0:80, 0:128], dbc_sb[0:80, c * TC:(c + 1) * TC],
                                     start=True, stop=True)
                    nc.scalar.copy(brep_sb[:, c * TC:(c + 1) * TC], pb[:])
                    pc2 = pstile()
                    nc.tensor.matmul(pc2[:], rep8_sb[0:80, 128:256], dbc_sb[0:80, c * TC:(c + 1) * TC],
                                     start=True, stop=True)
                    nc.scalar.copy(crep_sb[:, c * TC:(c + 1) * TC], pc2[:])

                # ---- scan block (full-L scans, LOOK-ahead pipelined ry reduce) ----
                yfs = [[None, None] for _ in range(4)]     # [c][half]
                for half in range(2):
                    py = [pytile() for _ in range(4)]
                    hcs = [None] * 16

                    def emit_ry(j, py=py, hcs=hcs):
                        for c in range(4):
                            nc.tensor.matmul(py[c][:], ry16_sb[:, j * 128:(j + 1) * 128],
                                             hcs[j][:, c * TC:(c + 1) * TC],
                                             start=(j == 0), stop=(j == 15),
                                             skip_group_check=True)

                    for j in range(16):
                        g = half * 16 + j
                        dA = btile()
                        dBx = btile()
                        for c in range(4):
                            pdtr = pstile()
                            nc.tensor.matmul(pdtr[:], r16_sb[:, j * 128:(j + 1) * 128],
                                             dt_sb[:, half * L + c * TC: half * L + (c + 1) * TC],
                                             start=True, stop=True)
                            nc.scalar.activation(dA[:, c * TC:(c + 1) * TC], pdtr[:], AF.Exp,
                                                 bias=0.0, scale=lc[:, 12 + g: 13 + g])
                            pdxr = pstile()
                            nc.tensor.matmul(pdxr[:], r16_sb[:, j * 128:(j + 1) * 128],
                                             dtxc[half][:, c * TC:(c + 1) * TC],
                                             start=True, stop=True)
                            nc.vector.tensor_tensor(dBx[:, c * TC:(c + 1) * TC], pdxr[:],
                                                    brep_sb[:, c * TC:(c + 1) * TC], OP.mult)
                        h = btile()
                        nc.vector.tensor_tensor_scan(h[:], dA[:], dBx[:], 0.0, OP.mult, OP.add)
                        hc = btile()
                        nc.gpsimd.tensor_tensor(hc[:], h[:], crep_sb[:], OP.mult)
                        hcs[j] = hc
                        if j >= LOOK:
                            emit_ry(j - LOOK)
                    for j in range(16 - LOOK, 16):
                        emit_ry(j)

                    for c in range(4):
                        ty = gtile()
                        nc.vector.scalar_tensor_tensor(ty[:],
                                                       xc_sb[:, half * L + c * TC: half * L + (c + 1) * TC],
                                                       lc[:, 44 + half: 45 + half], py[c][:],
                                                       OP.mult, OP.add)
                        yf = gtile(TC, BF16)
                        nc.vector.tensor_tensor(yf[:], ty[:],
                                                zs_sb[:, half * L + c * TC: half * L + (c + 1) * TC],
                                                OP.mult)
                        yfs[c][half] = yf

                # ---- out_proj partials -> DRAM -> AllReduce ----
                for c in range(4):
                    for mt in range(4):
                        po = pstile()
                        nc.tensor.matmul(po[:], outw_t[0][:, mt * 128:(mt + 1) * 128], yfs[c][0][:],
                                         start=True, stop=False)
                        nc.tensor.matmul(po[:], outw_t[1][:, mt * 128:(mt + 1) * 128], yfs[c][1][:],
                                         start=False, stop=True)
                        oe = gtile(TC, BF16)
                        nc.scalar.copy(oe[:], po[:])
                        nc.sync.dma_start(xarb[c][mt * 128:(mt + 1) * 128, :], oe[:])
                    if _NO_COLL:
                        nc.sync.dma_start(xaro[c][:], xarb[c][:])
                    else:
                        nc.gpsimd.collective_compute("AllReduce", mybir.AluOpType.add,
                                                     replica_groups=GROUPS,
                                                     ins=[xarb[c].opt()], outs=[xaro[c].opt()])
                    for kt in range(4):
                        nc.sync.dma_start(x_sb[:, kt * L + c * TC: kt * L + (c + 1) * TC],
                                          xaro[c][kt * 128:(kt + 1) * 128, :])
                    if _DBG_LAYER == l:
                        nc.sync.dma_start(t_xdbg[:, c * TC:(c + 1) * TC], xaro[c][:])

            # =================== HEAD ===================
            fcw_sb = []
            for i in range(2):
                ft = pwtile(512)
                nc.sync.dma_start(ft[:], t_fcw[:, i * 512:(i + 1) * 512])
                fcw_sb.append(ft)
            for mt2 in range(2):
                pl_ = pstile(128, 1)
                for kt in range(4):
                    idx = kt * 2 + mt2
                    nc.tensor.matmul(pl_[:], fcw_sb[idx // 4][:, (idx % 4) * 128:(idx % 4 + 1) * 128],
                                     x_sb[:, kt * L + (L - 1): kt * L + L],
                                     start=(kt == 0), stop=(kt == 3))
                lg = gtile(1, F32)
                nc.scalar.activation(lg[:, 0:1], pl_[:], AF.Identity,
                                     bias=fcb_sb[:, mt2:mt2 + 1], scale=1.0)
                nc.sync.dma_start(t_logits[mt2 * 128:(mt2 + 1) * 128, 0:1], lg[:, 0:1])

    nc.compile()
    return nc


def _get_prog():
    global _PROG
    if _PROG is None:
        _PROG = _build()
    return _PROG


_LAST_RES = None


def kernel(**inputs):
    global _LAST_RES
    import os
    nc = _get_prog()
    from concourse.bass_utils import run_bass_kernel_spmd
    in_maps = _prep(inputs)
    trace = os.environ.get('KTRACE', '') == '1'
    res = run_bass_kernel_spmd(nc, in_maps, list(range(NCOR)), trace=trace)
    _LAST_RES = res
    out = np.stack([res.results[0]['logits'][:, 0], res.results[4]['logits'][:, 0]])
    return out.astype(np.float32)


# revision 12
# speedup vs baseline: 1.0251x; 1.0251x over previous
"""Trainium2 Bass kernel for a Mamba-based byte LLM.

Sharding: 8 cores = 2 batch groups (quads) x 4-way tensor-parallel split of
d_inner (256 channels/core). Frontend (embed + 2 convs + LNs) is sequence
sharded (512 tokens/core) then all-gathered within each quad. Each Mamba
layer: in_proj/conv/scan/out_proj channel-sharded; x_proj partials and
out_proj partials all-reduced within the quad.

Selective scan uses the hardware tensor_tensor_scan (state = a*state + b along
the free dim) in an "s-minor" layout: partition p = (dd, s) with s = p % 16,
8 d-channels x 16 states per 128-partition tile; time on the free dim.

v2: all-bf16 dataflow (weights, activations, collectives; PSUM stays f32),
full-length 2048-col scans (no chunk handoff), Softplus fusion, gpsimd
offload for dtxc/hc, software-pipelined scan block (PE replication matmuls
run LOOK tiles ahead of the DVE scans).
"""
import sys
import numpy as np

sys.path.insert(0, '/opt/trn_rl_repo')

VOCAB = 256; EMB = 384; DM = 512; DI = 1024; DS = 16; DC = 4; DTR = 32
NL = 8; BATCH = 2; L = 2048
NCOR = 8; NQ = 4; DLOC = 256          # d_inner channels per core
TC = 512                              # time chunk
EXT = 520                             # frontend token window (512 + 2*4 halo)
LLOC = 512                            # frontend tokens per core
GEN_BUFS = 20
BIG_BUFS = 14
PW_BUFS = 8
LOOK = 4                              # scan-block ry lookahead (pipeline depth)

_PROG = None
_DBG_LAYER = None     # set to an int l before first kernel() call to also dump x after layer l
_SIM_COMPAT = False   # replace Gelu/Silu with sim-implemented functions (debug only)
_NL_RUN = NL          # number of layers to emit (debug only)
_NO_COLL = False      # replace collectives with local DMA copies (debug only)


def _bf16():
    import ml_dtypes
    return ml_dtypes.bfloat16


def _f32(x):
    return np.ascontiguousarray(np.asarray(x), dtype=np.float32)


def _bf(x):
    return np.ascontiguousarray(np.asarray(x, dtype=np.float32).astype(_bf16()))


def _pack_lhsT(W):
    """W: (M, K) with M,K multiples of 128 -> flat (128, KT*MT*128),
    tile (kt, mt) at cols [(kt*MT+mt)*128 : +128], tile[p, m] = W[128mt+m, 128kt+p]."""
    M, K = W.shape
    KT, MT = K // 128, M // 128
    out = np.zeros((128, KT * MT * 128), np.float32)
    for kt in range(KT):
        for mt in range(MT):
            blk = W[mt * 128:(mt + 1) * 128, kt * 128:(kt + 1) * 128].T
            out[:, (kt * MT + mt) * 128:(kt * MT + mt + 1) * 128] = blk
    return out


def _prep(inp):
    tokens = np.asarray(inp['tokens'])
    emb = _f32(inp['emb'])
    c1w = _f32(inp['conv1_w']); c1b = _f32(inp['conv1_b'])
    g1 = _f32(inp['ln1_g']); b1 = _f32(inp['ln1_b'])
    c2w = _f32(inp['conv2_w']); c2b = _f32(inp['conv2_b'])
    g2 = _f32(inp['ln2_g']); b2 = _f32(inp['ln2_b'])
    pos = _f32(inp['pos_emb'])
    inw = _f32(inp['in_proj_w']); cw = _f32(inp['conv_w']); cb = _f32(inp['conv_b'])
    xpw = _f32(inp['x_proj_w']); dtw = _f32(inp['dt_proj_w']); dtb = _f32(inp['dt_proj_b'])
    alog = _f32(inp['A_log']); Dp = _f32(inp['D'])
    outw = _f32(inp['out_proj_w'])
    fcw = _f32(inp['fc_w']); fcb = _f32(inp['fc_b'])

    # ---- shared tensors ----
    shared = {}
    shared['ones_row'] = _bf(np.ones((1, 128), np.float32))
    fe = np.zeros((128, 28), np.float32)
    fe[:, 0] = np.arange(128); fe[:, 1] = 1.0; fe[:, 26] = 1e-5
    for mt in range(4):
        fe[:, 2 + mt] = c1b[mt * 128:(mt + 1) * 128]
        fe[:, 6 + mt] = g1[mt * 128:(mt + 1) * 128]
        fe[:, 10 + mt] = b1[mt * 128:(mt + 1) * 128]
        fe[:, 14 + mt] = c2b[mt * 128:(mt + 1) * 128]
        fe[:, 18 + mt] = g2[mt * 128:(mt + 1) * 128]
        fe[:, 22 + mt] = b2[mt * 128:(mt + 1) * 128]
    shared['fe_cols'] = fe
    shared['ones_col'] = _bf(np.ones((128, 1), np.float32))
    shared['embw'] = _bf(emb)                              # (256, 384) lhsT as-is
    # conv1: tile (tap, kt, mt): [p, idx*128+m] = c1w[128mt+m, 128kt+p, tap]
    c1f = np.zeros((128, 5 * 3 * 4 * 128), np.float32)
    for tap in range(5):
        for kt in range(3):
            for mt in range(4):
                idx = (tap * 3 + kt) * 4 + mt
                c1f[:, idx * 128:(idx + 1) * 128] = c1w[mt * 128:(mt + 1) * 128,
                                                        kt * 128:(kt + 1) * 128, tap].T
    shared['c1w_flat'] = _bf(c1f)
    c2f = np.zeros((128, 3 * 4 * 4 * 128), np.float32)
    for tap in range(3):
        for kt in range(4):
            for mt in range(4):
                idx = (tap * 4 + kt) * 4 + mt
                c2f[:, idx * 128:(idx + 1) * 128] = c2w[mt * 128:(mt + 1) * 128,
                                                        kt * 128:(kt + 1) * 128, tap].T
    shared['c2w_flat'] = _bf(c2f)
    r16 = np.zeros((128, 16 * 128), np.float32)
    ry16 = np.zeros((128, 16 * 128), np.float32)
    p_idx = np.arange(128)
    for j in range(16):
        for m in range(128):
            r16[8 * j + m // 16, j * 128 + m] = 1.0
    for j in range(16):
        for k in range(128):
            ry16[k, j * 128 + 8 * j + k // 16] = 1.0
    shared['R16'] = _bf(r16)
    shared['RY16'] = _bf(ry16)
    # repBC: lhsT patterns that broadcast dbc's B rows (32:48) / C rows (64:80)
    # to all 128 partitions (s = p % 16), contracting over the full 80 rows.
    repbc = np.zeros((128, 256), np.float32)
    for p in range(128):
        repbc[32 + p % 16, p] = 1.0
        repbc[64 + p % 16, 128 + p] = 1.0
    shared['repBC'] = _bf(repbc)
    shared['fcw_flat'] = _bf(_pack_lhsT(fcw))              # (128, 8*128)
    fcb_c = np.zeros((128, 2), np.float32)
    fcb_c[:, 0] = fcb[0:128]; fcb_c[:, 1] = fcb[128:256]
    shared['fcb_cols'] = fcb_c

    # ---- per-core tensors ----
    in_maps = []
    for core in range(NCOR):
        b, q = core // NQ, core % NQ
        my = slice(DLOC * q, DLOC * (q + 1))
        m = dict(shared)

        t0 = LLOC * q
        te = np.full((1, EXT), -1.0, np.float32)
        for i in range(EXT):
            t = t0 - 4 + i
            if 0 <= t < L:
                te[0, i] = float(tokens[b, t])
        m['tok_ext'] = _bf(te)
        fm = np.zeros((128, 516), np.float32)
        for j in range(516):
            t = t0 - 2 + j
            if 0 <= t < L:
                fm[:, j] = 1.0
        m['fe_mask'] = fm
        pl = np.zeros((128, 4 * 512), np.float32)
        for mt in range(4):
            pl[:, mt * 512:(mt + 1) * 512] = pos[t0:t0 + 512, mt * 128:(mt + 1) * 128].T
        m['pos_loc'] = pl

        inw_f = np.zeros((NL, 128, 2048), np.float32)
        outw_f = np.zeros((NL, 128, 1024), np.float32)
        xpw_f = np.zeros((NL, 128, 160), np.float32)
        dtw_f = np.zeros((NL, 128, 256), np.float32)
        lcols = np.zeros((NL, 128, 48), np.float32)
        convdiag = np.zeros((NL, 128, 1024), np.float32)
        for l in range(NL):
            Wl = np.concatenate([inw[l, my, :], inw[l, DI + DLOC * q: DI + DLOC * (q + 1), :]], 0)
            inw_f[l] = _pack_lhsT(Wl)                       # (512,512) -> (128,2048)
            outw_f[l] = _pack_lhsT(outw[l][:, my])          # (512,256) -> (128,1024)
            Wxp = np.zeros((80, DLOC), np.float32)
            Wxp[0:32] = xpw[l, 0:32, my]
            Wxp[32:48] = xpw[l, 32:48, my]
            Wxp[64:80] = xpw[l, 48:64, my]
            for kt in range(2):
                xpw_f[l][:, kt * 80:(kt + 1) * 80] = Wxp[:, kt * 128:(kt + 1) * 128].T
            Wdt = dtw[l, my, :]                             # (256, 32)
            for mt in range(2):
                dtw_f[l][0:32, mt * 128:(mt + 1) * 128] = Wdt[mt * 128:(mt + 1) * 128, :].T
            for half in range(2):
                hs = slice(half * 128, (half + 1) * 128)
                for tap in range(DC):
                    idx = half * 4 + tap
                    convdiag[l][:, idx * 128:(idx + 1) * 128] = np.diag(cw[l, my, :][hs.start:hs.stop, tap])
                lcols[l, :, 8 + half] = cb[l, my][hs]
                lcols[l, :, 10 + half] = dtb[l, my][hs]
                lcols[l, :, 44 + half] = Dp[l, my][hs]
            A = -np.exp(alog[l, my, :])                     # (256, 16)
            for g in range(32):
                lcols[l, :, 12 + g] = A[8 * g + p_idx // 16, p_idx % 16]
        m['inw_flat'] = _bf(inw_f); m['outw_flat'] = _bf(outw_f); m['xpw_flat'] = _bf(xpw_f)
        m['dtw_flat'] = _bf(dtw_f); m['lcols'] = lcols; m['convdiag'] = _bf(convdiag)
        in_maps.append(m)
    return in_maps


def _build():
    import concourse.bass as bass
    import concourse.bacc as bacc
    import concourse.mybir as mybir
    import concourse.tile as tile

    F32 = mybir.dt.float32
    BF16 = mybir.dt.bfloat16
    AF = mybir.ActivationFunctionType
    OP = mybir.AluOpType
    GROUPS = [[0, 1, 2, 3], [4, 5, 6, 7]]
    AF_GELU = AF.Tanh if _SIM_COMPAT else AF.Gelu
    AF_SILU = AF.Sigmoid if _SIM_COMPAT else AF.Silu

    nc = bacc.Bacc("TRN2", target_bir_lowering=False, debug=False, num_devices=NCOR)

    def din(name, shape, dt_=BF16):
        return nc.dram_tensor(name, list(shape), dt_, kind="ExternalInput").ap()

    t_tok = din('tok_ext', (1, EXT))
    t_ones = din('ones_row', (1, 128))
    t_onesc = din('ones_col', (128, 1))
    t_fe = din('fe_cols', (128, 28), F32)
    t_embw = din('embw', (256, 384))
    t_c1w = din('c1w_flat', (128, 7680))
    t_c2w = din('c2w_flat', (128, 6144))
    t_r16 = din('R16', (128, 2048))
    t_ry16 = din('RY16', (128, 2048))
    t_rep8 = din('repBC', (128, 256))
    t_fcw = din('fcw_flat', (128, 1024))
    t_fcb = din('fcb_cols', (128, 2), F32)
    t_pos = din('pos_loc', (128, 2048), F32)
    t_femask = din('fe_mask', (128, 516), F32)
    t_inw = din('inw_flat', (NL, 128, 2048))
    t_outw = din('outw_flat', (NL, 128, 1024))
    t_xpw = din('xpw_flat', (NL, 128, 160))
    t_dtw = din('dtw_flat', (NL, 128, 256))
    t_lcols = din('lcols', (NL, 128, 48), F32)
    t_cdiag = din('convdiag', (NL, 128, 1024))
    t_logits = nc.dram_tensor("logits", [256, 1], F32, kind="ExternalOutput").ap()
    t_xdbg = (nc.dram_tensor("xdbg", [512, L], BF16, kind="ExternalOutput").ap()
              if _DBG_LAYER is not None else None)

    with tile.TileContext(nc) as tc:
        with tc.tile_pool(name="per", bufs=1) as per, \
             tc.tile_pool(name="gen", bufs=GEN_BUFS) as gen, \
             tc.tile_pool(name="big", bufs=BIG_BUFS) as bigp, \
             tc.tile_pool(name="pw", bufs=PW_BUFS) as pw, \
             tc.tile_pool(name="lcp", bufs=2) as lcp, \
             tc.tile_pool(name="ps", bufs=4, space="PSUM") as ps, \
             tc.tile_pool(name="psy", bufs=4, space="PSUM") as psy, \
             tc.tile_pool(name="dram", bufs=1, space="DRAM") as dram:

            cnt = [0]

            def gtile(cols=TC, dt_=BF16):
                cnt[0] += 1
                return gen.tile([128, cols], dt_, tag="g", name=f"g{cnt[0]}")

            def btile():
                cnt[0] += 1
                return bigp.tile([128, L], BF16, tag="b", name=f"b{cnt[0]}")

            def pwtile(cols=512, dt_=BF16, rows=128):
                cnt[0] += 1
                return pw.tile([rows, cols], dt_, tag="w", name=f"w{cnt[0]}")

            def pstile(rows=128, cols=TC):
                cnt[0] += 1
                return ps.tile([rows, cols], F32, tag="ps", name=f"p{cnt[0]}")

            def pytile():
                cnt[0] += 1
                return psy.tile([128, TC], F32, tag="py", name=f"y{cnt[0]}")

            # ---- persistent consts ----
            tok_sb = per.tile([1, EXT], BF16, tag="tok")
            nc.sync.dma_start(tok_sb[:], t_tok[:])
            ones_sb = per.tile([1, 128], BF16, tag="ones")
            nc.sync.dma_start(ones_sb[:], t_ones[:])
            onesc_sb = per.tile([128, 1], BF16, tag="onesc")
            nc.sync.dma_start(onesc_sb[:], t_onesc[:])
            fe_sb = per.tile([128, 28], F32, tag="fec")
            nc.sync.dma_start(fe_sb[:], t_fe[:])
            r16_sb = per.tile([128, 2048], BF16, tag="r16")
            nc.sync.dma_start(r16_sb[:], t_r16[:])
            ry16_sb = per.tile([128, 2048], BF16, tag="ry16")
            nc.sync.dma_start(ry16_sb[:], t_ry16[:])
            rep8_sb = per.tile([128, 256], BF16, tag="rep8")
            nc.sync.dma_start(rep8_sb[:], t_rep8[:])
            fcb_sb = per.tile([128, 2], F32, tag="fcb")
            nc.sync.dma_start(fcb_sb[:], t_fcb[:])
            fem_sb = per.tile([128, 516], F32, tag="fem")
            nc.sync.dma_start(fem_sb[:], t_femask[:])

            iota = fe_sb[:, 0:1]

            # ---- persistent big tensors ----
            x_sb = per.tile([128, 4 * L], BF16, tag="xsb")         # K-tile kt at cols kt*L
            xi_sb = per.tile([128, 2 * (L + 3)], BF16, tag="xi")   # half at cols h*(L+3), 3-zero pad
            z3_sb = per.tile([128, 4], BF16, tag="z3")
            nc.vector.memset(z3_sb[:], 0.0)
            nc.scalar.copy(xi_sb[:, 0:3], z3_sb[:, 0:3])
            nc.scalar.copy(xi_sb[:, L + 3: L + 6], z3_sb[:, 0:3])
            zs_sb = per.tile([128, 2 * L], BF16, tag="zs")          # silu(z)
            xc_sb = per.tile([128, 2 * L], BF16, tag="xc")
            dt_sb = per.tile([128, 2 * L], BF16, tag="dt")
            dbc_sb = per.tile([80, L], BF16, tag="dbc")
            brep_sb = per.tile([128, L], BF16, tag="brep")
            crep_sb = per.tile([128, L], BF16, tag="crep")

            # ---- DRAM bounces ----
            x0b = dram.tile([512, 512], BF16, tag="x0b")
            xg = dram.tile([4, 512, 512], BF16, tag="xg")
            dbcb = dram.tile([80, L], BF16, tag="dbcb")
            dbcr = dram.tile([80, L], BF16, tag="dbcr")
            xarb = dram.tile([512, L], BF16, tag="xarb")
            xaro = dram.tile([512, L], BF16, tag="xaro")

            # =================== FRONTEND ===================
            # one-hot: (256, 520) as 2 row-tiles
            oh = []
            for kt in range(2):
                oht = gtile(EXT, BF16)
                for nch in range(2):
                    ptok = pstile(128, 260)
                    nc.tensor.matmul(ptok[:], ones_sb[:], tok_sb[:, nch * 260:(nch + 1) * 260],
                                     start=True, stop=True)
                    if kt == 0:
                        nc.vector.tensor_scalar(oht[:, nch * 260:(nch + 1) * 260],
                                                ptok[:], iota, None, OP.is_equal)
                    else:
                        nc.vector.tensor_scalar(oht[:, nch * 260:(nch + 1) * 260],
                                                ptok[:], 128.0, iota, OP.subtract, OP.is_equal)
                oh.append(oht)
            # embedding: x_emb (384, 520) = embw.T @ oh
            embw_sb = []
            for kt in range(2):
                et = pwtile(384)
                nc.sync.dma_start(et[:], t_embw[kt * 128:(kt + 1) * 128, :])
                embw_sb.append(et)
            xemb = []
            for mt in range(3):
                xt = gtile(EXT, BF16)
                for nch in range(2):
                    pe = pstile(128, 260)
                    for kt in range(2):
                        nc.tensor.matmul(pe[:], embw_sb[kt][:, mt * 128:(mt + 1) * 128],
                                         oh[kt][:, nch * 260:(nch + 1) * 260],
                                         start=(kt == 0), stop=(kt == 1))
                    nc.scalar.copy(xt[:, nch * 260:(nch + 1) * 260], pe[:])
                xemb.append(xt)

            def fe_conv(src, KT, TAPS, wflat, bias_base, n_out, src_off):
                """conv + gelu: out tiles (128, n_out) list of 4. src: list of KT tiles.
                out col j corresponds to src col j + src_off + tap."""
                nhalf = n_out // 2
                outs = [gtile(EXT, BF16) for _ in range(4)]
                for nch in range(2):
                    pcs = [pstile(128, nhalf) for _ in range(4)]
                    for tap in range(TAPS):
                        for kt in range(KT):
                            base = (tap * KT + kt) * 4 * 128
                            wt = pwtile(512)
                            nc.sync.dma_start(wt[:], wflat[:, base:base + 512])
                            for mt in range(4):
                                col = src_off + tap + nch * nhalf
                                nc.tensor.matmul(pcs[mt][:], wt[:, mt * 128:(mt + 1) * 128],
                                                 src[kt][:, col:col + nhalf],
                                                 start=(tap == 0 and kt == 0),
                                                 stop=(tap == TAPS - 1 and kt == KT - 1))
                    for mt in range(4):
                        nc.scalar.activation(outs[mt][:, nch * nhalf:(nch + 1) * nhalf], pcs[mt][:],
                                             AF_GELU, bias=fe_sb[:, bias_base + mt:bias_base + mt + 1],
                                             scale=1.0)
                return outs

            # conv1: out ext range [2, 518) -> 516 cols; src col = out_col + tap (out col 0 == ext 2)
            h1 = fe_conv(xemb, 3, 5, t_c1w, 2, 516, 0)

            def fe_ln(src, n, g_base, b_base, mask=None):
                """layernorm over 512 channels (4 tiles); returns 4 tiles (128, n)."""
                nhalf = n // 2
                sq = []
                for mt in range(4):
                    st = gtile(EXT, BF16)
                    nc.scalar.activation(st[:, 0:n], src[mt][:, 0:n], AF.Square)
                    sq.append(st)
                mu = gtile(EXT, BF16)     # row 0: mean
                m2 = gtile(EXT, F32)
                var = gtile(EXT, F32)
                std = gtile(EXT, F32)
                rstd = gtile(EXT, BF16)
                for nch in range(2):
                    pm = pstile(1, nhalf)
                    pm2 = pstile(1, nhalf)
                    for mt in range(4):
                        nc.tensor.matmul(pm[:], onesc_sb[:, 0:1], src[mt][:, nch * nhalf:(nch + 1) * nhalf],
                                         start=(mt == 0), stop=(mt == 3))
                        nc.tensor.matmul(pm2[:], onesc_sb[:, 0:1], sq[mt][:, nch * nhalf:(nch + 1) * nhalf],
                                         start=(mt == 0), stop=(mt == 3))
                    sl = slice(nch * nhalf, (nch + 1) * nhalf)
                    nc.scalar.activation(mu[0:1, sl], pm[:], AF.Copy, bias=0.0, scale=1.0 / 512)
                    nc.scalar.activation(m2[0:1, sl], pm[:], AF.Square, bias=0.0, scale=1.0 / 512)
                    # var = pm2/512 - mu^2
                    nc.vector.scalar_tensor_tensor(var[0:1, sl], pm2[:], 1.0 / 512, m2[0:1, sl],
                                                   OP.mult, OP.subtract)
                    nc.scalar.activation(std[0:1, sl], var[0:1, sl], AF.Sqrt,
                                         bias=fe_sb[0:1, 26:27], scale=1.0)
                    with nc.allow_low_precision(reason="rstd for broadcast matmul"):
                        nc.vector.reciprocal(rstd[0:1, sl], std[0:1, sl])
                outs = []
                for mt in range(4):
                    ot = gtile(EXT, BF16)
                    for nch in range(2):
                        sl = slice(nch * nhalf, (nch + 1) * nhalf)
                        pmb = pstile(128, nhalf)
                        prb = pstile(128, nhalf)
                        nc.tensor.matmul(pmb[:], ones_sb[:], mu[0:1, sl], start=True, stop=True)
                        nc.tensor.matmul(prb[:], ones_sb[:], rstd[0:1, sl], start=True, stop=True)
                        d0 = gtile(EXT, F32)
                        nc.vector.tensor_tensor(d0[:, 0:nhalf], src[mt][:, sl], pmb[:], OP.subtract)
                        nc.vector.tensor_tensor(d0[:, 0:nhalf], d0[:, 0:nhalf], prb[:], OP.mult)
                        if mask is None:
                            nc.vector.tensor_scalar(ot[:, sl], d0[:, 0:nhalf],
                                                    fe_sb[:, g_base + mt:g_base + mt + 1],
                                                    fe_sb[:, b_base + mt:b_base + mt + 1],
                                                    OP.mult, OP.add)
                        else:
                            nc.vector.tensor_scalar(d0[:, 0:nhalf], d0[:, 0:nhalf],
                                                    fe_sb[:, g_base + mt:g_base + mt + 1],
                                                    fe_sb[:, b_base + mt:b_base + mt + 1],
                                                    OP.mult, OP.add)
                            nc.vector.tensor_tensor(ot[:, sl], d0[:, 0:nhalf], mask[:, sl], OP.mult)
                    outs.append(ot)
                return outs

            xn1 = fe_ln(h1, 516, 6, 10, mask=fem_sb)
            # conv2: out ext [4, 516) -> 512 cols; xn1 col c == ext 2+c; src col = out_col + 1 + tap
            h2 = fe_conv(xn1, 4, 3, t_c2w, 14, 512, 1)
            xn2 = fe_ln(h2, 512, 18, 22)
            # + pos_emb -> x0 tiles; DMA to bounce
            for mt in range(4):
                pt = pwtile(512, F32)
                nc.sync.dma_start(pt[:], t_pos[:, mt * 512:(mt + 1) * 512])
                x0t = gtile(512, BF16)
                nc.vector.tensor_tensor(x0t[:], xn2[mt][:, 0:512], pt[:], OP.add)
                nc.sync.dma_start(x0b[mt * 128:(mt + 1) * 128, :], x0t[:])
            if _NO_COLL:
                for q in range(4):
                    nc.sync.dma_start(xg[q], x0b[:])
            else:
                nc.gpsimd.collective_compute("AllGather", mybir.AluOpType.bypass,
                                             replica_groups=GROUPS, ins=[x0b.opt()], outs=[xg.opt()])
            for kt in range(4):
                for q in range(4):
                    nc.sync.dma_start(x_sb[:, kt * L + q * 512: kt * L + (q + 1) * 512],
                                      xg[q, kt * 128:(kt + 1) * 128, :])

            # =================== LAYERS ===================
            for l in range(_NL_RUN):
                lc = lcp.tile([128, 48], F32, tag="lc")
                nc.sync.dma_start(lc[:], t_lcols[l])
                inw_t = []
                for kt in range(4):
                    wt = pwtile(512)
                    nc.sync.dma_start(wt[:], t_inw[l][:, kt * 512:(kt + 1) * 512])
                    inw_t.append(wt)
                xpw_t = pwtile(160)
                nc.sync.dma_start(xpw_t[:], t_xpw[l])
                dtw_t = pwtile(256)
                nc.sync.dma_start(dtw_t[:], t_dtw[l])
                outw_t = []
                for kt in range(2):
                    wt = pwtile(512)
                    nc.sync.dma_start(wt[:], t_outw[l][:, kt * 512:(kt + 1) * 512])
                    outw_t.append(wt)

                # ---- in_proj ----
                for c in range(4):
                    for mt in range(4):
                        px = pstile()
                        for kt in range(4):
                            nc.tensor.matmul(px[:], inw_t[kt][:, mt * 128:(mt + 1) * 128],
                                             x_sb[:, kt * L + c * TC: kt * L + (c + 1) * TC],
                                             start=(kt == 0), stop=(kt == 3))
                        if mt < 2:
                            nc.scalar.copy(xi_sb[:, mt * (L + 3) + 3 + c * TC: mt * (L + 3) + 3 + (c + 1) * TC],
                                           px[:])
                        else:
                            nc.scalar.activation(zs_sb[:, (mt - 2) * L + c * TC: (mt - 2) * L + (c + 1) * TC],
                                                 px[:], AF_SILU)

                # ---- causal depthwise conv (PE diag matmuls) + silu ----
                cdiag_t = []
                for half in range(2):
                    ct = pwtile(512)
                    nc.sync.dma_start(ct[:], t_cdiag[l][:, half * 512:(half + 1) * 512])
                    cdiag_t.append(ct)
                for half in range(2):
                    for c in range(4):
                        base = half * (L + 3) + c * TC
                        pcv = pstile()
                        for tap in range(DC):
                            nc.tensor.matmul(pcv[:], cdiag_t[half][:, tap * 128:(tap + 1) * 128],
                                             xi_sb[:, base + tap: base + tap + TC],
                                             start=(tap == 0), stop=(tap == DC - 1))
                        nc.scalar.activation(xc_sb[:, half * L + c * TC: half * L + (c + 1) * TC],
                                             pcv[:], AF_SILU,
                                             bias=lc[:, 8 + half: 9 + half], scale=1.0)

                # ---- x_proj partial -> DRAM -> AllReduce ----
                for c in range(4):
                    pd = pstile(80, TC)
                    for kt in range(2):
                        nc.tensor.matmul(pd[:], xpw_t[:, kt * 80:(kt + 1) * 80],
                                         xc_sb[:, kt * L + c * TC: kt * L + (c + 1) * TC],
                                         start=(kt == 0), stop=(kt == 1))
                    cnt[0] += 1
                    de = gen.tile([80, TC], BF16, tag="g", name=f"g{cnt[0]}")
                    nc.scalar.copy(de[:], pd[:])
                    nc.sync.dma_start(dbcb[:, c * TC:(c + 1) * TC], de[:])
                if _NO_COLL:
                    nc.sync.dma_start(dbcr[:], dbcb[:])
                else:
                    nc.gpsimd.collective_compute("AllReduce", mybir.AluOpType.add,
                                                 replica_groups=GROUPS, ins=[dbcb.opt()], outs=[dbcr.opt()])
                nc.sync.dma_start(dbc_sb[:], dbcr[:])

                # ---- dt = softplus(dtw @ dbc[0:32] + b) ----
                for c in range(4):
                    for mt in range(2):
                        pdt = pstile()
                        nc.tensor.matmul(pdt[:], dtw_t[0:32, mt * 128:(mt + 1) * 128],
                                         dbc_sb[0:32, c * TC:(c + 1) * TC], start=True, stop=True)
                        edt = gtile(TC, F32)
                        nc.scalar.activation(edt[:], pdt[:], AF.Exp,
                                             bias=lc[:, 10 + mt: 11 + mt], scale=1.0)
                        nc.scalar.activation(dt_sb[:, mt * L + c * TC: mt * L + (c + 1) * TC],
                                             edt[:], AF.Ln, bias=1.0, scale=1.0)

                # ---- dtxc (gpsimd), B/C broadcast for full L ----
                dtxc = []
                for half in range(2):
                    dx = btile()
                    nc.gpsimd.tensor_tensor(dx[:],
                                            dt_sb[:, half * L: (half + 1) * L],
                                            xc_sb[:, half * L: (half + 1) * L],
                                            OP.mult)
                    dtxc.append(dx)
                for c in range(4):
                    pb = pstile()
                    nc.tensor.matmul(pb[:], rep8_sb[0:80, 0:128], dbc_sb[0:80, c * TC:(c + 1) * TC],
                                     start=True, stop=True)
                    nc.scalar.copy(brep_sb[:, c * TC:(c + 1) * TC], pb[:])
                    pc2 = pstile()
                    nc.tensor.matmul(pc2[:], rep8_sb[0:80, 128:256], dbc_sb[0:80, c * TC:(c + 1) * TC],
                                     start=True, stop=True)
                    nc.scalar.copy(crep_sb[:, c * TC:(c + 1) * TC], pc2[:])

                # ---- scan block (full-L scans, LOOK-ahead pipelined ry reduce) ----
                yfs = [[None, None] for _ in range(4)]     # [c][half]
                for half in range(2):
                    py = [pytile() for _ in range(4)]
                    hcs = [None] * 16

                    def emit_ry(j, py=py, hcs=hcs):
                        for c in range(4):
                            nc.tensor.matmul(py[c][:], ry16_sb[:, j * 128:(j + 1) * 128],
                                             hcs[j][:, c * TC:(c + 1) * TC],
                                             start=(j == 0), stop=(j == 15),
                                             skip_group_check=True)

                    for j in range(16):
                        g = half * 16 + j
                        dA = btile()
                        dBx = btile()
                        for c in range(4):
                            pdtr = pstile()
                            nc.tensor.matmul(pdtr[:], r16_sb[:, j * 128:(j + 1) * 128],
                                             dt_sb[:, half * L + c * TC: half * L + (c + 1) * TC],
                                             start=True, stop=True)
                            nc.scalar.activation(dA[:, c * TC:(c + 1) * TC], pdtr[:], AF.Exp,
                                                 bias=0.0, scale=lc[:, 12 + g: 13 + g])
                            pdxr = pstile()
                            nc.tensor.matmul(pdxr[:], r16_sb[:, j * 128:(j + 1) * 128],
                                             dtxc[half][:, c * TC:(c + 1) * TC],
                                             start=True, stop=True)
                            nc.vector.tensor_tensor(dBx[:, c * TC:(c + 1) * TC], pdxr[:],
                                                    brep_sb[:, c * TC:(c + 1) * TC], OP.mult)
                        h = btile()
                        nc.vector.tensor_tensor_scan(h[:], dA[:], dBx[:], 0.0, OP.mult, OP.add)
                        hc = btile()
                        nc.gpsimd.tensor_tensor(hc[:], h[:], crep_sb[:], OP.mult)
                        hcs[j] = hc
                        if j >= LOOK:
                            emit_ry(j - LOOK)
                    for j in range(16 - LOOK, 16):
                        emit_ry(j)

                    for c in range(4):
                        ty = gtile()
                        nc.vector.scalar_tensor_tensor(ty[:],
                                                       xc_sb[:, half * L + c * TC: half * L + (c + 1) * TC],
                                                       lc[:, 44 + half: 45 + half], py[c][:],
                                                       OP.mult, OP.add)
                        yf = gtile(TC, BF16)
                        nc.vector.tensor_tensor(yf[:], ty[:],
                                                zs_sb[:, half * L + c * TC: half * L + (c + 1) * TC],
                                                OP.mult)
                        yfs[c][half] = yf

                # ---- out_proj partials -> DRAM -> AllReduce ----
                for c in range(4):
                    for mt in range(4):
                        po = pstile()
                        nc.tensor.matmul(po[:], outw_t[0][:, mt * 128:(mt + 1) * 128], yfs[c][0][:],
                                         start=True, stop=False)
                        nc.tensor.matmul(po[:], outw_t[1][:, mt * 128:(mt + 1) * 128], yfs[c][1][:],
                                         start=False, stop=True)
                        oe = gtile(TC, BF16)
                        nc.scalar.copy(oe[:], po[:])
                        nc.sync.dma_start(xarb[mt * 128:(mt + 1) * 128, c * TC:(c + 1) * TC], oe[:])

                if _NO_COLL:
                    nc.sync.dma_start(xaro[:], xarb[:])
                else:
                    nc.gpsimd.collective_compute("AllReduce", mybir.AluOpType.add,
                                                 replica_groups=GROUPS, ins=[xarb.opt()], outs=[xaro.opt()])
                for kt in range(4):
                    nc.sync.dma_start(x_sb[:, kt * L:(kt + 1) * L],
                                      xaro[kt * 128:(kt + 1) * 128, :])
                if _DBG_LAYER == l:
                    nc.sync.dma_start(t_xdbg[:], xaro[:])

            # =================== HEAD ===================
            fcw_sb = []
            for i in range(2):
                ft = pwtile(512)
                nc.sync.dma_start(ft[:], t_fcw[:, i * 512:(i + 1) * 512])
                fcw_sb.append(ft)
            for mt2 in range(2):
                pl_ = pstile(128, 1)
                for kt in range(4):
                    idx = kt * 2 + mt2
                    nc.tensor.matmul(pl_[:], fcw_sb[idx // 4][:, (idx % 4) * 128:(idx % 4 + 1) * 128],
                                     x_sb[:, kt * L + (L - 1): kt * L + L],
                                     start=(kt == 0), stop=(kt == 3))
                lg = gtile(1, F32)
                nc.scalar.activation(lg[:, 0:1], pl_[:], AF.Identity,
                                     bias=fcb_sb[:, mt2:mt2 + 1], scale=1.0)
                nc.sync.dma_start(t_logits[mt2 * 128:(mt2 + 1) * 128, 0:1], lg[:, 0:1])

    nc.compile()
    return nc


def _get_prog():
    global _PROG
    if _PROG is None:
        _PROG = _build()
    return _PROG


_LAST_RES = None


def kernel(**inputs):
    global _LAST_RES
    import os
    nc = _get_prog()
    from concourse.bass_utils import run_bass_kernel_spmd
    in_maps = _prep(inputs)
    trace = os.environ.get('KTRACE', '') == '1'
    res = run_bass_kernel_spmd(nc, in_maps, list(range(NCOR)), trace=trace)
    _LAST_RES = res
    out = np.stack([res.results[0]['logits'][:, 0], res.results[4]['logits'][:, 0]])
    return out.astype(np.float32)
